# revision 23
# baseline (speedup 1.0000x reference)
"""CNSN (eval-mode CrossNorm+SelfNorm) Trainium2 kernel.

Reference computation (per sample b, channel c over spatial HW):
    mean, std  (unbiased std over the 4096 spatial elements)
    gate_m = sigmoid(MLP_m([mean, std]))      # Linear(2,16)+ReLU+Linear(16,1)
    gate_s = sigmoid(MLP_s([mean, std]))
    out = (x - m)/s * (s*gate_s) + m*gate_m
        = x * gate_s + m * (gate_m - gate_s)   # per-channel affine

Strategy: pure data-parallel over batch (64 samples -> 8 per core), bf16 I/O
(host converts f32->bf16; ~2^-9 relative rounding vs the 2e-2 gate) to halve
the HBM traffic that bounds this kernel (~358 GB/s per NeuronCore).
Per core: 8 DMA steps of [128 rows, 8192] bf16 "fat rows" (2 complete
channels per SBUF partition row = 16KB contiguous DMA lines; x is declared
[8, 128, 8192], the same linear bytes as [16, 128, 4096]).
Per step: bn_stats/bn_aggr (DVE) -> tiny fused MLP (DVE+ACT) gates ->
the per-channel affine applied by ACT (first 2048 cols of each half) and
DVE tensor_scalar (last 2048; dual-op mult+add, 2 elem/cycle on bf16) ->
DMA out. Apply+store are emitted lag=2 steps behind load+stats+gates
(software pipeline) so the big ACT/DVE ops never stall on the fresh
cross-engine gate chain. Memory-bound: 16 MiB in + 16 MiB out per core.
"""

import ml_dtypes
import numpy as np

import concourse.bass as bass
import concourse.tile as tile
from concourse import mybir
from concourse.bass_utils import run_bass_kernel_spmd

F32 = mybir.dt.float32
BF16 = mybir.dt.bfloat16
NP_BF16 = ml_dtypes.bfloat16
AF = mybir.ActivationFunctionType
ALU = mybir.AluOpType

N_CORES = 8
B, C, H, W = 64, 256, 64, 64
HW = H * W                     # 4096
B_PER_CORE = B // N_CORES      # 8
TILES = B_PER_CORE * C // 128  # 16 tiles of [128, HW] per core
EPS = 1e-5
# bn_aggr returns population variance (M2/n); torch-style unbiased var is
# M2/(n-1), so std = sqrt(var_pop * n/(n-1) + eps).
VAR_CORR = HW / (HW - 1)

# consts layout, one [128, 130] f32 tensor (all rows identical):
#   [:,   0: 32] W10  = concat(wm1, ws1)[:, 0]   (weight on the mean input)
#   [:,  32: 64] W11  = concat(wm1, ws1)[:, 1]   (weight on the std input)
#   [:,  64: 96] B1   = concat(bm1, bs1)
#   [:,  96:112] W2M  = wm2[0]
#   [:, 112:128] W2S  = ws2[0]
#   [:, 128:129] B2M  = bm2[0]
#   [:, 129:130] B2S  = bs2[0]
N_CONST = 130
DMA_BATCH = 2  # tiles per DMA step; must match _build_nc(batch=) default

_CACHE: dict = {}
LAST_RESULTS = None  # BassKernelResults of the most recent run (for profiling)


def _split_excess_waits(nc: bass.Bass) -> int:
    """Move surplus sync waits onto standalone nops.

    The TPB EVENTS field encodes exactly ONE wait per hardware instruction
    (see NEURON_ISA_TPB_EVENTS); walrus codegen hard-fails with "Too many
    sync wait commands" when Tile attaches more. Sequencers execute
    same-engine instructions in program order, so hoisting all but one wait
    onto nofuse nops placed immediately before the instruction preserves
    semantics.
    """
    builder_of = {
        mybir.EngineType.DVE: nc.vector,
        mybir.EngineType.Activation: nc.scalar,
        mybir.EngineType.PE: nc.tensor,
        mybir.EngineType.Pool: nc.gpsimd,
        mybir.EngineType.SP: nc.sync,
    }
    n_split = 0
    for bb in nc.main_func.blocks:
        insts = bb.instructions
        out = []
        changed = False
        for ins in list(insts):
            si = ins.sync_info
            if si is not None and si.on_wait and len(si.on_wait) > 1:
                assert si.on_update is None or len(si.on_update) <= 1, ins
                waits = list(si.on_wait)
                for w in waits[:-1]:
                    nop = builder_of[ins.engine].nop(nofuse=True).ins
                    # the builder appended it to some (current) block; yank it
                    for b2 in nc.main_func.blocks:
                        try:
                            b2.instructions.remove(nop)
                            break
                        except ValueError:
                            pass
                    nop.sync_info = mybir.SyncInfo(on_wait=[w], on_update=[])
                    out.append(nop)
                ins.sync_info = mybir.SyncInfo(
                    on_wait=[waits[-1]], on_update=list(si.on_update or [])
                )
                changed = True
                n_split += 1
            out.append(ins)
        if changed:
            insts.clear()
            insts.extend(out)
    return n_split


def _build_nc(
    repeat: int = 1,
    indep: bool = False,
    dma_only: bool = False,
    load_engines: tuple = ("sync",),
    store_engines: tuple = ("gpsimd",),
    batch: int = DMA_BATCH,
    lag: int = 2,
    xin_bufs: int = 6,
    yout_bufs: int = 3,
    apply_dve: int = 2048,
) -> bass.Bass:
    """Build the per-core Bass program.

    repeat > 1 (odd) chains N tile sweeps inside one NEFF. indep=False: each
    sweep reads the previous sweep's output (x -> y -> scratch -> y -> ...);
    indep=True: every sweep reads x and writes y (no cross-sweep RAW, so
    sweeps pipeline — steady-state slope without boundary-drain penalty).
    Used only by timing.py: per-sweep HW time = slope of wall time between
    two repeat values, which cancels the multi-ms axon dispatch overhead.

    dma_only: skip all compute; store the loaded tile back (DMA roofline probe).
    load_engines/store_engines: DMA queue assignment; steps are split along
    the free axis into len(engines) chunks, one per queue ("sync"/"scalar"
    HWDGE rings, "gpsimd" SWDGE ring).

    batch: tiles per DMA step. The DRAM tensors are declared
    [TILES/batch, 128, batch*HW] — the same linear bytes — so each SBUF
    partition row holds `batch` complete channels as one contiguous
    16KB(batch=2) DMA line: bigger descriptors, half the DMA instructions.
    Stats/gates/apply are done per HW-sized half-row.

    lag: software-pipeline depth in steps. The apply+store for step k are
    emitted alongside the load/stats/gates for step k+lag, so the ACT
    engine's big apply ops and the DVE's bn_stats never wait on the
    fresh cross-engine gate chain (which ping-pongs DVE<->ACT).
    """
    assert repeat % 2 == 1, "odd repeat keeps the final sweep writing y"
    assert TILES % batch == 0
    steps = TILES // batch
    fhw = batch * HW
    nc = bass.Bass()
    x = nc.declare_dram_parameter("x", [steps, 128, fhw], BF16, isOutput=False)
    cn = nc.declare_dram_parameter("consts", [128, N_CONST], F32, isOutput=False)
    y = nc.declare_dram_parameter("y", [steps, 128, fhw], BF16, isOutput=True)
    scratch = (
        nc.dram_tensor("scratch", [steps, 128, fhw], BF16)
        if (repeat > 1 and not indep)
        else None
    )
    eng = {"sync": nc.sync, "scalar": nc.scalar, "gpsimd": nc.gpsimd}

    def split_dma(engines, dst_tile, src_ap, sb_is_out):
        n = len(engines)
        step = fhw // n
        for j, e in enumerate(engines):
            sl = slice(j * step, (j + 1) * step)
            if sb_is_out:
                eng[e].dma_start(out=dst_tile[:, sl], in_=src_ap[:, sl])
            else:
                eng[e].dma_start(out=src_ap[:, sl], in_=dst_tile[:, sl])

    with tile.TileContext(nc) as tc:
        with (
            tc.tile_pool(name="consts", bufs=1) as consts,
            tc.tile_pool(name="xin", bufs=xin_bufs) as xin,
            tc.tile_pool(name="yout", bufs=yout_bufs) as yout,
            tc.tile_pool(name="gatep", bufs=2 * batch * (lag + 2)) as gatep,
            tc.tile_pool(name="statp", bufs=4) as statp,
            tc.tile_pool(name="small", bufs=8 * batch * (lag + 2)) as small,
        ):
            cst0 = consts.tile([128, N_CONST], F32)
            nc.sync.dma_start(out=cst0[:], in_=cn[:, :])
            # Bounce through DVE so every DVE consumer of the constants
            # depends on a same-engine product: the consts-DMA wait then
            # lives on this copy (TensorCopy has spare sync-wait slots)
            # instead of a TensorScalarPtr, whose encoding has only one.
            cst = consts.tile([128, N_CONST], F32)
            nc.vector.tensor_copy(out=cst[:], in_=cst0[:])
            eps_t = consts.tile([128, 1], F32)
            nc.vector.memset(eps_t[:], EPS)
            w10t = cst[:, 0:32]
            w11t = cst[:, 32:64]
            b1t = cst[:, 64:96]
            w2mt = cst[:, 96:112]
            w2st = cst[:, 112:128]
            b2mt = cst[:, 128:129]
            b2st = cst[:, 129:130]

            def front(src, k):
                """Load step k and compute its per-half gates.

                Returns (xt, [(bc, gate_s) per half]) for back()."""
                xt = xin.tile([128, fhw], BF16)
                split_dma(load_engines, xt, src[k, :, :], True)
                if dma_only:
                    return xt, None
                # One ACT pre-touch absorbs the x-load DMA wait so no
                # later ACT op needs it (the encoding has 2 wait slots).
                pre = small.tile([128, 1], F32)
                nc.scalar.activation(out=pre[:], in_=xt[:, 0:1], func=AF.Copy)

                # mean / population-variance per half-row (one channel each)
                stats = statp.tile(
                    [128, batch * (HW // 512), nc.vector.BN_STATS_DIM], F32
                )
                xv = xt[:].rearrange("p (a b) -> p a b", b=512)
                for s in range(batch * (HW // 512)):
                    nc.vector.bn_stats(out=stats[:, s, :], in_=xv[:, s, :])
                gates = []
                for hh in range(batch):
                    mv = small.tile([128, nc.vector.BN_AGGR_DIM], F32)
                    nc.vector.bn_aggr(
                        out=mv[:],
                        in_=stats[:, hh * (HW // 512) : (hh + 1) * (HW // 512), :],
                    )
                    mean = mv[:, 0:1]
                    # std = sqrt(var_pop * n/(n-1) + eps)
                    sd = small.tile([128, 1], F32)
                    nc.scalar.activation(
                        out=sd[:], in_=mv[:, 1:2], func=AF.Sqrt, bias=eps_t[:],
                        scale=VAR_CORR,
                    )
                    # layer 1 (both MLPs fused, 32 hidden units total):
                    # h = relu(mean*W10 + std*W11 + B1)
                    t1 = small.tile([128, 32], F32)
                    nc.vector.tensor_scalar_mul(out=t1[:], in0=w10t, scalar1=mean)
                    t2 = small.tile([128, 32], F32)
                    nc.vector.tensor_scalar_mul(out=t2[:], in0=w11t, scalar1=sd[:])
                    h = small.tile([128, 32], F32)
                    nc.vector.tensor_add(out=h[:], in0=t1[:], in1=t2[:])
                    nc.vector.tensor_add(out=h[:], in0=h[:], in1=b1t)
                    nc.vector.tensor_scalar_max(out=h[:], in0=h[:], scalar1=0.0)
                    # layer 2: gate = sigmoid(h . w2 + b2), per branch
                    hw2 = small.tile([128, 32], F32)
                    nc.vector.tensor_mul(out=hw2[:], in0=h[:], in1=cst[:, 96:128])
                    gm = small.tile([128, 1], F32)
                    nc.vector.reduce_sum(
                        out=gm[:], in_=hw2[:, 0:16], axis=mybir.AxisListType.X
                    )
                    gs = small.tile([128, 1], F32)
                    nc.vector.reduce_sum(
                        out=gs[:], in_=hw2[:, 16:32], axis=mybir.AxisListType.X
                    )
                    gate_m = small.tile([128, 1], F32)
                    nc.scalar.activation(
                        out=gate_m[:], in_=gm[:], func=AF.Sigmoid, bias=b2mt,
                        scale=1.0,
                    )
                    gate_s = gatep.tile([128, 1], F32)
                    nc.scalar.activation(
                        out=gate_s[:], in_=gs[:], func=AF.Sigmoid, bias=b2st,
                        scale=1.0,
                    )
                    # bias_c = (gate_m - gate_s) * mean
                    bc = gatep.tile([128, 1], F32)
                    nc.vector.tensor_sub(out=bc[:], in0=gate_m[:], in1=gate_s[:])
                    nc.vector.tensor_mul(out=bc[:], in0=bc[:], in1=mean)
                    gates.append((bc, gate_s))
                return xt, gates

            def back(dst, k, xt, gates):
                """Apply + store step k (lag steps after its front)."""
                if dma_only:
                    split_dma(store_engines, xt, dst[k, :, :], False)
                    return
                yt = yout.tile([128, fhw], BF16)
                # ACT pre-touch absorbs the y-slot store-WAR DMA wait.
                nc.scalar.activation(out=yt[:, 0:1], in_=xt[:, 0:1], func=AF.Copy)
                for hh, (bc, gate_s) in enumerate(gates):
                    base = hh * HW
                    cut = HW - apply_dve
                    # out = gate_s * x + bias_c  (ACT does [0:cut]; DVE
                    # offloads the tail so neither engine is the bottleneck)
                    nc.scalar.activation(
                        out=yt[:, base : base + cut],
                        in_=xt[:, base : base + cut],
                        func=AF.Identity, bias=bc[:], scale=gate_s[:],
                    )
                    if apply_dve:
                        nc.vector.tensor_scalar(
                            out=yt[:, base + cut : base + HW],
                            in0=xt[:, base + cut : base + HW],
                            scalar1=gate_s[:], scalar2=bc[:],
                            op0=ALU.mult, op1=ALU.add,
                        )
                split_dma(store_engines, yt, dst[k, :, :], False)

            for r in range(repeat):
              if indep:
                src, dst = x, y
              else:
                src = x if r == 0 else (y if r % 2 == 1 else scratch)
                dst = y if r % 2 == 0 else scratch
              inflight = {}
              for k in range(steps + lag):
                if k < steps:
                    inflight[k] = front(src, k)
                if k >= lag:
                    xt, gates = inflight.pop(k - lag)
                    back(dst, k - lag, xt, gates)
    _split_excess_waits(nc)
    nc.finalize()
    return nc


def _pack_consts(wm1, bm1, wm2, bm2, ws1, bs1, ws2, bs2) -> np.ndarray:
    w1 = np.concatenate([wm1, ws1], axis=0).astype(np.float32)  # [32, 2]
    b1 = np.concatenate([bm1, bs1], axis=0).astype(np.float32)  # [32]
    row = np.concatenate(
        [
            w1[:, 0], w1[:, 1], b1,
            wm2[0].astype(np.float32), ws2[0].astype(np.float32),
            bm2.astype(np.float32).reshape(1), bs2.astype(np.float32).reshape(1),
        ]
    )
    assert row.shape == (N_CONST,)
    return np.ascontiguousarray(np.broadcast_to(row, (128, N_CONST))).astype(np.float32)


def make_in_maps(x, wm1, bm1, wm2, bm2, ws1, bs1, ws2, bs2):
    """Shard full inputs into per-core in_maps (host-side prep)."""
    x = np.asarray(x, dtype=np.float32)
    assert x.shape == (B, C, H, W)
    consts = _pack_consts(wm1, bm1, wm2, bm2, ws1, bs1, ws2, bs2)
    # f32 -> bf16 on host: halves HBM traffic on device (the memory
    # roofline); bf16 rounding is ~2^-9 relative, far under the 2e-2 gate.
    xb = x.astype(NP_BF16)
    in_maps = []
    for c in range(N_CORES):
        xs = np.ascontiguousarray(
            xb[c * B_PER_CORE : (c + 1) * B_PER_CORE]
        ).reshape(TILES // DMA_BATCH, 128, DMA_BATCH * HW)
        in_maps.append({"x": xs, "consts": consts})
    return in_maps


def kernel(x, wm1, bm1, wm2, bm2, ws1, bs1, ws2, bs2):
    global LAST_RESULTS
    if "nc" not in _CACHE:
        _CACHE["nc"] = _build_nc()
    nc = _CACHE["nc"]

    in_maps = make_in_maps(x, wm1, bm1, wm2, bm2, ws1, bs1, ws2, bs2)

    res = run_bass_kernel_spmd(nc, in_maps, list(range(N_CORES)))
    LAST_RESULTS = res
    y = np.concatenate(
        [
            res.results[c]["y"].astype(np.float32).reshape(B_PER_CORE, C, H, W)
            for c in range(N_CORES)
        ],
        axis=0,
    )
    return np.ascontiguousarray(y, dtype=np.float32)

